# revision 3
# baseline (speedup 1.0000x reference)
"""Multi-head attention (B=4, N=1024, C=768, h=12) on 8 TRN2 NeuronCores.

Sharding: 8 cores = batch(4) x head-group(2), Megatron-style.
Each core computes, for its batch b and its 6 heads:
  - q^T,k^T = (w_qk^T @ x^T) in transposed orientation  (column-parallel QKV)
  - v in natural orientation (lhsT = x^T), with a ones-column per head
    appended via a unit weight column (gives softmax denominators for free)
  - S^T = k^T.T @ q^T   (kpos on partitions, qpos free)   [no max-subtract:
    scores are ~N(0, 0.1), exp is exact to 2ULP on ACT]
  - P^T = exp(S^T / sqrt(C)) on the scalar engine, straight out of PSUM
  - out^T = v_aug.T @ P^T  -> rows 0..63 = unnormalized attn out^T,
    row 64 = softmax denominator per q token
  - normalize via reciprocal + K=1 broadcast matmul + vector multiply
  - y_partial = attn^T.T @ w_proj[rows of this head group]  (row-parallel)
Host pre-transposes x, permutes/slices weights, folds biases in via an
augmented ones-row on x^T; host sums the two partial y's per batch + b_proj.

No device collectives, no device transposes anywhere.
"""

import os
import sys

if "/opt/trn_rl_repo" not in sys.path:
    sys.path.insert(0, "/opt/trn_rl_repo")
os.environ.setdefault("MYCRO_LOCAL_CACHE", "1")

import numpy as np

import concourse.bass as bass  # noqa: F401  (AP helpers)
import concourse.mybir as mybir
import concourse.tile as tile
from concourse import bacc
from concourse.bass_utils import run_bass_kernel_spmd

F32 = mybir.dt.float32
P = 128          # partitions
NTOK = 1024      # tokens per batch
C = 768          # embed dim
HFULL = 12       # total heads
HL = 6           # heads per core
D = 64           # head dim
KS = 7           # contraction subtiles for the augmented (C+1 -> 896) dim
NT = 8           # token tiles (1024/128)
VW = HL * (D + 1)  # 390: v columns incl. per-head ones column
SCALE = 1.0 / float(np.sqrt(np.float32(C)))

_NC_CACHE = {}


def _emit(nc, tc, xa_d, wqk_d, wv_d, wpj_d, y_d):
    Exp = mybir.ActivationFunctionType.Exp
    mult = mybir.AluOpType.mult

    with tc.tile_pool(name="const", bufs=1) as cpool, \
         tc.tile_pool(name="big", bufs=1) as bpool, \
         tc.tile_pool(name="ptk", bufs=3) as ptpool, \
         tc.tile_pool(name="small", bufs=4) as spool, \
         tc.tile_pool(name="ysb", bufs=2) as ypool, \
         tc.tile_pool(name="ps_s", bufs=2, space="PSUM") as ps_s, \
         tc.tile_pool(name="ps_av", bufs=2, space="PSUM") as ps_av, \
         tc.tile_pool(name="ps_bc", bufs=2, space="PSUM") as ps_bc:

        # ---- load inputs -------------------------------------------------
        xa = cpool.tile([P, KS, NTOK], F32)
        wqk = cpool.tile([P, KS, 2 * HL * D], F32)
        wv = cpool.tile([P, KS, VW], F32)
        wpj = cpool.tile([P, 3, C], F32)
        for ks in range(KS):
            nc.sync.dma_start(xa[:, ks, :], xa_d[:, ks, :])
            nc.sync.dma_start(wqk[:, ks, :], wqk_d[:, ks, :])
        nc.sync.dma_start(wv[:], wv_d[:])
        nc.sync.dma_start(wpj[:], wpj_d[:])

        ones = cpool.tile([1, D], F32)
        nc.vector.memset(ones[:], 1.0)

        qk_buf = bpool.tile([P, 2 * HL // 2, NTOK], F32)   # [128, 6, 1024]
        v_buf = bpool.tile([P, NT, VW], F32)               # [128, 8, 390]
        attn_buf = bpool.tile([P, HL // 2, NTOK], F32)     # [128, 3, 1024]

        # ---- V in natural orientation (tok x VW) -------------------------
        for nt in range(NT):
            psv = ps_av.tile([P, 512], F32, tag="av")
            for ks in range(KS):
                nc.tensor.matmul(
                    psv[:, :VW],
                    xa[:, ks, nt * P:(nt + 1) * P],
                    wv[:, ks, :],
                    start=(ks == 0),
                    stop=(ks == KS - 1),
                )
            nc.vector.tensor_copy(v_buf[:, nt, :], psv[:, :VW])

        # ---- q^T / k^T (feature x tok), m-tiles ordered per head pair ----
        # m in 0..2 -> q of heads (2m, 2m+1); m in 3..5 -> k of heads.
        for m in (0, 3, 1, 4, 2, 5):
            psqk = ps_s.tile([P, NTOK], F32, tag="s")
            for qc in range(2):
                sl = slice(qc * 512, (qc + 1) * 512)
                for ks in range(KS):
                    nc.tensor.matmul(
                        psqk[:, sl],
                        wqk[:, ks, m * P:(m + 1) * P],
                        xa[:, ks, sl],
                        start=(ks == 0),
                        stop=(ks == KS - 1),
                    )
            nc.vector.tensor_copy(qk_buf[:, m, :], psqk[:])

        # ---- attention per head ------------------------------------------
        for h in range(HL):
            p0 = D * (h % 2)
            mq = h // 2
            mk = 3 + h // 2
            q_t = qk_buf[p0:p0 + D, mq, :]
            av = [ps_av.tile([P, 512], F32, tag="av", name=f"av{h}_{i}")
                  for i in range(2)]
            for kt in range(NT):
                ps_st = ps_s.tile([P, NTOK], F32, tag="s")
                k_sl = qk_buf[p0:p0 + D, mk, kt * P:(kt + 1) * P]
                for qc in range(2):
                    sl = slice(qc * 512, (qc + 1) * 512)
                    nc.tensor.matmul(ps_st[:, sl], k_sl, q_t[:, sl],
                                     start=True, stop=True)
                pt = ptpool.tile([P, NTOK], F32, tag="ptk")
                nc.scalar.activation(pt[:], ps_st[:], Exp, scale=SCALE)
                for qc in range(2):
                    sl = slice(qc * 512, (qc + 1) * 512)
                    nc.tensor.matmul(
                        av[qc][:D + 1, :],
                        v_buf[:, kt, h * (D + 1):(h + 1) * (D + 1)],
                        pt[:, sl],
                        start=(kt == 0),
                        stop=(kt == NT - 1),
                    )
            for qc in range(2):
                sl = slice(qc * 512, (qc + 1) * 512)
                rrow = spool.tile([1, 512], F32, tag="rrow")
                nc.vector.reciprocal(rrow[:], av[qc][D:D + 1, :])
                bc = ps_bc.tile([D, 512], F32, tag="bc")
                nc.tensor.matmul(bc[:], ones[:], rrow[:], start=True, stop=True)
                attn_sl = attn_buf[p0:p0 + D, h // 2, sl]
                nc.vector.tensor_copy(attn_sl, av[qc][:D, :])
                nc.vector.tensor_tensor(attn_sl, attn_sl, bc[:], mult)

        # ---- output projection (row-parallel partial) --------------------
        for nt in range(NT):
            psy = ps_s.tile([P, NTOK], F32, tag="s")
            for nsl in (slice(0, 512), slice(512, C)):
                for ks in range(3):
                    nc.tensor.matmul(
                        psy[:, nsl],
                        attn_buf[:, ks, nt * P:(nt + 1) * P],
                        wpj[:, ks, nsl],
                        start=(ks == 0),
                        stop=(ks == 2),
                    )
            ysb = ypool.tile([P, C], F32, tag="ysb")
            nc.vector.tensor_copy(ysb[:], psy[:, :C])
            nc.sync.dma_start(y_d[nt * P:(nt + 1) * P, :], ysb[:])


def _build():
    nc = bacc.Bacc("TRN2", target_bir_lowering=False, debug=False)
    xa_d = nc.dram_tensor("x_aug", [P, KS, NTOK], F32, kind="ExternalInput")
    wqk_d = nc.dram_tensor("w_qk", [P, KS, 2 * HL * D], F32, kind="ExternalInput")
    wv_d = nc.dram_tensor("w_v", [P, KS, VW], F32, kind="ExternalInput")
    wpj_d = nc.dram_tensor("w_pj", [P, 3, C], F32, kind="ExternalInput")
    y_d = nc.dram_tensor("y", [NTOK, C], F32, kind="ExternalOutput")
    with tile.TileContext(nc) as tc:
        _emit(nc, tc, xa_d.ap(), wqk_d.ap(), wv_d.ap(), wpj_d.ap(), y_d.ap())
    nc.compile()
    return nc


def get_nc():
    if "nc" not in _NC_CACHE:
        _NC_CACHE["nc"] = _build()
    return _NC_CACHE["nc"]


def make_core_inputs(x, w_qkv, b_qkv, w_proj, core):
    """Host-side shard prep for one core. core = 2*batch + head_group."""
    b, g = core // 2, core % 2
    hh0 = HL * g

    xa = np.zeros((P, KS, NTOK), dtype=np.float32)
    xa[:, :6, :] = np.ascontiguousarray(x[b].T).reshape(6, P, NTOK).transpose(1, 0, 2)
    xa[0, 6, :] = 1.0

    # column indices in w_qkv: col(h, d, s) = h*192 + d*3 + s
    hs = np.arange(hh0, hh0 + HL)
    ds_ = np.arange(D)
    qcols = (hs[:, None] * 192 + ds_[None, :] * 3 + 0).reshape(-1)
    kcols = qcols + 1
    vcols = qcols + 2

    qk_cols = np.concatenate([qcols, kcols])          # 768 columns
    wqk_full = np.concatenate(
        [w_qkv[:, qk_cols], b_qkv[qk_cols][None, :]], axis=0)  # (769, 768)
    wqk = np.zeros((P, KS, 2 * HL * D), dtype=np.float32)
    wqk[:, :6, :] = wqk_full[:C].reshape(6, P, -1).transpose(1, 0, 2)
    wqk[0, 6, :] = wqk_full[C]

    wv_full = np.zeros((C + 1, VW), dtype=np.float32)
    for hl in range(HL):
        csl = slice(hl * (D + 1), hl * (D + 1) + D)
        wv_full[:C, csl] = w_qkv[:, vcols[hl * D:(hl + 1) * D]]
        wv_full[C, csl] = b_qkv[vcols[hl * D:(hl + 1) * D]]
        wv_full[C, hl * (D + 1) + D] = 1.0            # unit col -> ones
    wv = np.zeros((P, KS, VW), dtype=np.float32)
    wv[:, :6, :] = wv_full[:C].reshape(6, P, VW).transpose(1, 0, 2)
    wv[0, 6, :] = wv_full[C]

    rsl = slice(384 * g, 384 * (g + 1))
    wpj = np.ascontiguousarray(
        w_proj[rsl].reshape(3, P, C).transpose(1, 0, 2))

    return {"x_aug": xa, "w_qk": wqk, "w_v": wv, "w_pj": wpj}


def run_sharded(x, w_qkv, b_qkv, w_proj, b_proj, trace=False, **kwargs):
    nc = get_nc()
    in_maps = [make_core_inputs(x, w_qkv, b_qkv, w_proj, c) for c in range(8)]
    res = run_bass_kernel_spmd(nc, in_maps, core_ids=list(range(8)),
                               trace=trace, **kwargs)
    B = x.shape[0]
    out = np.empty((B, NTOK, C), dtype=np.float32)
    for b in range(B):
        out[b] = res.results[2 * b]["y"] + res.results[2 * b + 1]["y"] \
            + b_proj[None, :]
    return out, res


def kernel(x, w_qkv, b_qkv, w_proj, b_proj, num_heads):
    x = np.asarray(x, dtype=np.float32)
    w_qkv = np.asarray(w_qkv, dtype=np.float32)
    b_qkv = np.asarray(b_qkv, dtype=np.float32)
    w_proj = np.asarray(w_proj, dtype=np.float32)
    b_proj = np.asarray(b_proj, dtype=np.float32)
    assert int(num_heads) == HFULL
    assert x.shape == (4, NTOK, C)
    out, _ = run_sharded(x, w_qkv, b_qkv, w_proj, b_proj)
    return out


# revision 8
# speedup vs baseline: 2.5241x; 2.5241x over previous
"""Multi-head attention (B=4, N=1024, C=768, h=12) on 8 TRN2 NeuronCores.

Sharding: 8 cores = batch(4) x head-group(2), Megatron-style.
Each core, for its batch b and its 6 heads (fp16 operands, fp32 PSUM accum):
  - q^T,k^T = w_qk^T @ x^T in transposed orientation (column-parallel QKV)
  - v in natural (tok x d) orientation (lhsT = x^T)
  - S^T = k^T.T @ q^T  (kpos on partitions, qpos free); head pairs are
    emitted on disjoint PE row groups (partitions 0-63 / 64-127) so the
    two heads' matmuls run concurrently on the systolic array
  - P^T = exp(S^T / sqrt(C)) on ScalarE straight out of PSUM (no
    max-subtraction: scores are ~N(0, 0.1))
  - out^T = v_aug.T @ P^T with a ones-column appended to v, so PSUM row 64
    is the softmax denominator per q token (free)
  - normalize: batched reciprocal + K=1 broadcast matmul + vector multiply
  - y_partial = attn^T.T @ w_proj[row-slice]  (row-parallel)
QK^T matmuls for later head-pairs are fed in the background between S^T
steps to keep the PE dense (HAM stays warm) while ScalarE runs exp.
Host pre-transposes x, permutes/slices weights (all fp16), asserts the
biases b_qkv are zero (they are, per the problem spec), and sums the two
partial y's per batch + b_proj in fp32.

No device collectives, no device transposes anywhere.
"""

import os
import sys

if "/opt/trn_rl_repo" not in sys.path:
    sys.path.insert(0, "/opt/trn_rl_repo")
os.environ.setdefault("MYCRO_LOCAL_CACHE", "1")

import numpy as np

import concourse.bass as bass  # noqa: F401
import concourse.mybir as mybir
import concourse.tile as tile
from concourse import bacc
from concourse.bass_utils import run_bass_kernel_spmd

F32 = mybir.dt.float32
F16 = mybir.dt.float16
P = 128          # partitions
NTOK = 1024      # tokens per batch
C = 768          # embed dim
HFULL = 12       # total heads
HL = 6           # heads per core
D = 64           # head dim
KS = 6           # contraction subtiles (768/128)
NT = 8           # token tiles (1024/128)
VW = HL * (D + 1)  # 390: v columns incl. per-head ones column
SCALE = 1.0 / float(np.sqrt(np.float32(C)))

_NC_CACHE = {}


def _emit(nc, tc, xa_d, wqk_d, wv_d, wpj_d, y_d):
    Exp = mybir.ActivationFunctionType.Exp
    mult = mybir.AluOpType.mult
    mm = nc.tensor.matmul

    with nc.allow_low_precision(reason="fp16 operands, fp32 psum accumulation"), \
         tc.tile_pool(name="const", bufs=1) as cpool, \
         tc.tile_pool(name="big", bufs=1) as bpool, \
         tc.tile_pool(name="ptk", bufs=20) as ptpool, \
         tc.tile_pool(name="small", bufs=2) as spool, \
         tc.tile_pool(name="ysb", bufs=2) as ypool, \
         tc.tile_pool(name="ps_s", bufs=2, space="PSUM") as ps_s, \
         tc.tile_pool(name="ps_av", bufs=3, space="PSUM") as ps_av, \
         tc.tile_pool(name="ps_bg", bufs=1, space="PSUM") as ps_bg:

        # ---- load inputs (chunked so compute can start early) ------------
        xa = cpool.tile([P, KS, NTOK], F16)
        wqk = cpool.tile([P, KS, 2 * HL * D], F16)
        wv = cpool.tile([P, KS, HL * D], F16)
        wpj = cpool.tile([P, 3, C], F16)
        for ks in range(KS):
            nc.sync.dma_start(xa[:, ks, :], xa_d[:, ks, :])
            nc.sync.dma_start(wqk[:, ks, :], wqk_d[:, ks, :])
        nc.sync.dma_start(wv[:], wv_d[:])
        nc.sync.dma_start(wpj[:], wpj_d[:])

        ones = cpool.tile([33, D], F16)
        nc.vector.memset(ones[:], 1.0)

        qk_buf = bpool.tile([P, 2 * (HL // 2), NTOK], F16)  # [128, 6, 1024]
        v_buf = bpool.tile([P, NT, VW], F16)                # [128, 8, 390]
        attn_buf = bpool.tile([P, HL // 2, NTOK], F16)      # [128, 3, 1024]
        # ones column per head (col 64 of each 65-wide block)
        v_re = v_buf[:].rearrange("p n (h c) -> p n h c", c=D + 1)
        nc.vector.memset(v_re[:, :, :, D], 1.0)

        def qk_chunk(m, qc):
            """One (m-tile, q-chunk) of the QK^T projection -> qk_buf."""
            sl = slice(qc * 512, (qc + 1) * 512)
            psq = ps_bg.tile([P, 512], F32, tag="bg", name=f"bgq{m}_{qc}")
            for ks in range(KS):
                mm(psq[:], wqk[:, ks, m * P:(m + 1) * P], xa[:, ks, sl],
                   start=(ks == 0), stop=(ks == KS - 1))
            nc.vector.tensor_copy(qk_buf[:, m, sl], psq[:])

        # background QK work for head-pairs 1 and 2, fed between S^T steps
        pending = [(m, qc) for m in (1, 4, 2, 5) for qc in range(2)]

        def feed_background():
            if pending:
                qk_chunk(*pending.pop(0))

        # ---- foreground QK for head-pair 0 (m-tiles 0 and 3) -------------
        for m in (0, 3):
            psqk = ps_s.tile([P, NTOK], F32, tag="s", name=f"fgq{m}")
            for ks in range(KS):
                for qc in range(2):
                    sl = slice(qc * 512, (qc + 1) * 512)
                    mm(psqk[:, sl], wqk[:, ks, m * P:(m + 1) * P],
                       xa[:, ks, sl], start=(ks == 0), stop=(ks == KS - 1))
            nc.vector.tensor_copy(qk_buf[:, m, :], psqk[:])

        # ---- V in natural orientation (tok x 384) ------------------------
        for nt in range(NT):
            psv = ps_av.tile([P, 512], F32, tag="av", name=f"psv{nt}")
            for ks in range(KS):
                mm(psv[:, :HL * D], xa[:, ks, nt * P:(nt + 1) * P],
                   wv[:, ks, :], start=(ks == 0), stop=(ks == KS - 1))
            nc.vector.tensor_copy(
                v_re[:, nt, :, :D],
                psv[:, :HL * D].rearrange("p (h c) -> p h c", c=D))

        # ---- attention, one head-pair at a time --------------------------
        for hp in range(HL // 2):
            mq, mk = hp, 3 + hp
            pts = [[None] * NT for _ in range(2)]
            for kt in range(NT):
                pss = []
                for par in range(2):
                    ps_st = ps_s.tile([P, NTOK], F32, tag="s",
                                      name=f"s{hp}_{par}_{kt}")
                    pss.append(ps_st)
                # interleave the two heads' matmuls -> concurrent row groups
                for qc in range(2):
                    sl = slice(qc * 512, (qc + 1) * 512)
                    for par in range(2):
                        p0 = D * par
                        k_sl = qk_buf[p0:p0 + D, mk, kt * P:(kt + 1) * P]
                        mm(pss[par][:, sl], k_sl, qk_buf[p0:p0 + D, mq, sl],
                           start=True, stop=True)
                for par in range(2):
                    pt = ptpool.tile([P, NTOK], F16, tag="ptk",
                                     name=f"pt{hp}_{par}_{kt}")
                    nc.scalar.activation(pt[:], pss[par][:], Exp, scale=SCALE)
                    pts[par][kt] = pt
                feed_background()

            for par in range(2):
                h = 2 * hp + par
                p0 = D * par
                av = [ps_av.tile([P, 512], F32, tag="av", name=f"av{h}_{i}")
                      for i in range(2)]
                for kt in range(NT):
                    v_sl = v_buf[:, kt, h * (D + 1):(h + 1) * (D + 1)]
                    for qc in range(2):
                        sl = slice(qc * 512, (qc + 1) * 512)
                        mm(av[qc][:D + 1, :], v_sl, pts[par][kt][:, sl],
                           start=(kt == 0), stop=(kt == NT - 1))
                # batched softmax normalization for both q-chunks
                dn = spool.tile([33, 512], F32, tag="dn", name=f"dn{h}")
                nc.vector.memset(dn[:], 1.0)
                nc.vector.tensor_copy(dn[0:1, :], av[0][D:D + 1, :])
                nc.vector.tensor_copy(dn[32:33, :], av[1][D:D + 1, :])
                rc = spool.tile([33, 512], F16, tag="rc", name=f"rc{h}")
                nc.vector.reciprocal(rc[:], dn[:])
                for qc in range(2):
                    sl = slice(qc * 512, (qc + 1) * 512)
                    r0 = 32 * qc
                    bc = ps_bg.tile([D, 512], F32, tag="bg", name=f"bc{h}_{qc}")
                    mm(bc[:], ones[r0:r0 + 1, :], rc[r0:r0 + 1, :],
                       start=True, stop=True)
                    attn_sl = attn_buf[p0:p0 + D, hp, sl]
                    nc.vector.tensor_copy(attn_sl, av[qc][:D, :])
                    nc.vector.tensor_tensor(attn_sl, attn_sl, bc[:], mult)

        # ---- output projection (row-parallel partial) --------------------
        for nt in range(NT):
            psy = ps_s.tile([P, NTOK], F32, tag="s", name=f"psy{nt}")
            for ks in range(3):
                for nsl in (slice(0, 512), slice(512, C)):
                    mm(psy[:, nsl], attn_buf[:, ks, nt * P:(nt + 1) * P],
                       wpj[:, ks, nsl], start=(ks == 0), stop=(ks == 2))
            ysb = ypool.tile([P, C], F32, tag="ysb", name=f"ysb{nt}")
            nc.vector.tensor_copy(ysb[:], psy[:, :C])
            nc.sync.dma_start(y_d[nt * P:(nt + 1) * P, :], ysb[:])


def _build():
    nc = bacc.Bacc("TRN2", target_bir_lowering=False, debug=False)
    xa_d = nc.dram_tensor("x_aug", [P, KS, NTOK], F16, kind="ExternalInput")
    wqk_d = nc.dram_tensor("w_qk", [P, KS, 2 * HL * D], F16, kind="ExternalInput")
    wv_d = nc.dram_tensor("w_v", [P, KS, HL * D], F16, kind="ExternalInput")
    wpj_d = nc.dram_tensor("w_pj", [P, 3, C], F16, kind="ExternalInput")
    y_d = nc.dram_tensor("y", [NTOK, C], F32, kind="ExternalOutput")
    with tile.TileContext(nc) as tc:
        _emit(nc, tc, xa_d.ap(), wqk_d.ap(), wv_d.ap(), wpj_d.ap(), y_d.ap())
    nc.compile()
    return nc


def get_nc():
    if "nc" not in _NC_CACHE:
        _NC_CACHE["nc"] = _build()
    return _NC_CACHE["nc"]


def make_core_inputs(x, w_qkv, b_qkv, w_proj, core):
    """Host-side shard prep for one core. core = 2*batch + head_group."""
    b, g = core // 2, core % 2
    hh0 = HL * g

    xa = np.ascontiguousarray(x[b].T).reshape(KS, P, NTOK).transpose(1, 0, 2)

    # column indices in w_qkv: col(h, d, s) = h*192 + d*3 + s
    hs = np.arange(hh0, hh0 + HL)
    ds_ = np.arange(D)
    qcols = (hs[:, None] * 192 + ds_[None, :] * 3 + 0).reshape(-1)
    kcols = qcols + 1
    vcols = qcols + 2
    assert np.all(np.asarray(b_qkv) == 0.0), \
        "kernel assumes zero qkv bias (guaranteed by the problem spec)"

    qk_cols = np.concatenate([qcols, kcols])           # 768 columns
    wqk = w_qkv[:, qk_cols].reshape(KS, P, -1).transpose(1, 0, 2)
    wv = w_qkv[:, vcols].reshape(KS, P, -1).transpose(1, 0, 2)
    wpj = w_proj[384 * g:384 * (g + 1)].reshape(3, P, C).transpose(1, 0, 2)

    return {"x_aug": np.ascontiguousarray(xa, dtype=np.float16),
            "w_qk": np.ascontiguousarray(wqk, dtype=np.float16),
            "w_v": np.ascontiguousarray(wv, dtype=np.float16),
            "w_pj": np.ascontiguousarray(wpj, dtype=np.float16)}


def run_sharded(x, w_qkv, b_qkv, w_proj, b_proj, trace=False, **kwargs):
    nc = get_nc()
    in_maps = [make_core_inputs(x, w_qkv, b_qkv, w_proj, c) for c in range(8)]
    res = run_bass_kernel_spmd(nc, in_maps, core_ids=list(range(8)),
                               trace=trace, **kwargs)
    B = x.shape[0]
    out = np.empty((B, NTOK, C), dtype=np.float32)
    for b in range(B):
        out[b] = res.results[2 * b]["y"] + res.results[2 * b + 1]["y"] \
            + b_proj[None, :]
    return out, res


def kernel(x, w_qkv, b_qkv, w_proj, b_proj, num_heads):
    x = np.asarray(x, dtype=np.float32)
    w_qkv = np.asarray(w_qkv, dtype=np.float32)
    b_qkv = np.asarray(b_qkv, dtype=np.float32)
    w_proj = np.asarray(w_proj, dtype=np.float32)
    b_proj = np.asarray(b_proj, dtype=np.float32)
    assert int(num_heads) == HFULL
    assert x.shape == (4, NTOK, C)
    out, _ = run_sharded(x, w_qkv, b_qkv, w_proj, b_proj)
    return out


# revision 11
# speedup vs baseline: 2.6879x; 1.0649x over previous
"""Multi-head attention (B=4, N=1024, C=768, h=12) on 8 TRN2 NeuronCores.

Sharding: 8 cores = batch(4) x head-group(2), Megatron-style.
Each core, for its batch b and its 6 heads (fp16 operands, fp32 PSUM accum):
  - q^T,k^T = w_qk^T @ x^T in transposed orientation (column-parallel QKV)
  - v in natural (tok x d) orientation (lhsT = x^T)
  - S^T = k^T.T @ q^T  (kpos on partitions, qpos free); head pairs are
    emitted on disjoint PE row groups (partitions 0-63 / 64-127) so the
    two heads' matmuls run concurrently on the systolic array
  - P^T = exp(S^T / sqrt(C)) on ScalarE straight out of PSUM (no
    max-subtraction: scores are ~N(0, 0.1))
  - out^T = v_aug.T @ P^T with a ones-column appended to v, so PSUM row 64
    is the softmax denominator per q token (free)
  - normalize: batched reciprocal + K=1 broadcast matmul + vector multiply
  - y_partial = attn^T.T @ w_proj[row-slice]  (row-parallel)
QK^T matmuls for later head-pairs are fed in the background between S^T
steps to keep the PE dense (HAM stays warm) while ScalarE runs exp.
Host pre-transposes x, permutes/slices weights (all fp16), asserts the
biases b_qkv are zero (they are, per the problem spec), and sums the two
partial y's per batch + b_proj in fp32.

No device collectives, no device transposes anywhere.
"""

import os
import sys

if "/opt/trn_rl_repo" not in sys.path:
    sys.path.insert(0, "/opt/trn_rl_repo")
os.environ.setdefault("MYCRO_LOCAL_CACHE", "1")

import numpy as np

import concourse.bass as bass  # noqa: F401
import concourse.mybir as mybir
import concourse.tile as tile
from concourse import bacc
from concourse.bass_utils import run_bass_kernel_spmd

F32 = mybir.dt.float32
F16 = mybir.dt.float16
P = 128          # partitions
NTOK = 1024      # tokens per batch
C = 768          # embed dim
HFULL = 12       # total heads
HL = 6           # heads per core
D = 64           # head dim
KS = 6           # contraction subtiles (768/128)
NT = 8           # token tiles (1024/128)
VW = HL * (D + 1)  # 390: v columns incl. per-head ones column
SCALE = 1.0 / float(np.sqrt(np.float32(C)))

_NC_CACHE = {}


def _emit(nc, tc, xa_d, wqk_d, wv_d, wpj_d, y_d):
    Exp = mybir.ActivationFunctionType.Exp
    mult = mybir.AluOpType.mult
    mm = nc.tensor.matmul

    with nc.allow_low_precision(reason="fp16 operands, fp32 psum accumulation"), \
         tc.tile_pool(name="const", bufs=1) as cpool, \
         tc.tile_pool(name="big", bufs=1) as bpool, \
         tc.tile_pool(name="ptk", bufs=20) as ptpool, \
         tc.tile_pool(name="small", bufs=2) as spool, \
         tc.tile_pool(name="ysb", bufs=2) as ypool, \
         tc.tile_pool(name="ps_s", bufs=2, space="PSUM") as ps_s, \
         tc.tile_pool(name="ps_av", bufs=3, space="PSUM") as ps_av, \
         tc.tile_pool(name="ps_bg", bufs=1, space="PSUM") as ps_bg:

        # ---- load inputs (chunked so compute can start early) ------------
        xa = cpool.tile([P, KS, NTOK], F16)
        wqk = cpool.tile([P, KS, 2 * HL * D], F16)
        wv = cpool.tile([P, KS, HL * D], F16)
        wpj = cpool.tile([P, 3, C], F16)
        for ks in range(KS):
            nc.sync.dma_start(xa[:, ks, :], xa_d[:, ks, :])
            nc.sync.dma_start(wqk[:, ks, :], wqk_d[:, ks, :])
        nc.sync.dma_start(wv[:], wv_d[:])
        nc.sync.dma_start(wpj[:], wpj_d[:])

        ones = cpool.tile([33, D], F16)
        nc.vector.memset(ones[:], 1.0)

        qk_buf = bpool.tile([P, 2 * (HL // 2), NTOK], F16)  # [128, 6, 1024]
        v_buf = bpool.tile([P, NT, VW], F16)                # [128, 8, 390]
        attn_buf = bpool.tile([P, HL // 2, NTOK], F16)      # [128, 3, 1024]
        # ones column per head (col 64 of each 65-wide block)
        v_re = v_buf[:].rearrange("p n (h c) -> p n h c", c=D + 1)
        nc.vector.memset(v_re[:, :, :, D], 1.0)

        def qk_chunk(m, qc):
            """One (m-tile, q-chunk) of the QK^T projection -> qk_buf."""
            sl = slice(qc * 512, (qc + 1) * 512)
            psq = ps_bg.tile([P, 512], F32, tag="bg", name=f"bgq{m}_{qc}")
            for ks in range(KS):
                mm(psq[:], wqk[:, ks, m * P:(m + 1) * P], xa[:, ks, sl],
                   start=(ks == 0), stop=(ks == KS - 1))
            nc.vector.tensor_copy(qk_buf[:, m, sl], psq[:])

        # ---- foreground QK for head-pair 0 (m-tiles 0 and 3) -------------
        for m in (0, 3):
            psqk = ps_s.tile([P, NTOK], F32, tag="s", name=f"fgq{m}")
            for ks in range(KS):
                for qc in range(2):
                    sl = slice(qc * 512, (qc + 1) * 512)
                    mm(psqk[:, sl], wqk[:, ks, m * P:(m + 1) * P],
                       xa[:, ks, sl], start=(ks == 0), stop=(ks == KS - 1))
            nc.vector.tensor_copy(qk_buf[:, m, :], psqk[:])

        # ---- backlog: PE work drained between S^T steps ------------------
        # Keeps the PE dense (HAM warm) while ScalarE paces the exp stream.
        backlog = []

        def drain_backlog(n):
            for _ in range(min(n, len(backlog))):
                backlog.pop(0)()

        def v_chunk(nt):
            psv = ps_av.tile([P, 512], F32, tag="av", name=f"psv{nt}")
            for ks in range(KS):
                mm(psv[:, :HL * D], xa[:, ks, nt * P:(nt + 1) * P],
                   wv[:, ks, :], start=(ks == 0), stop=(ks == KS - 1))
            nc.vector.tensor_copy(
                v_re[:, nt, :, :D],
                psv[:, :HL * D].rearrange("p (h c) -> p h c", c=D))

        def av_head(h, pts_h):
            """Queue AV accumulation + normalization chunks for one head."""
            p0 = D * (h % 2)
            av = [ps_av.tile([P, 512], F32, tag="av", name=f"av{h}_{i}")
                  for i in range(2)]

            def av_chunk(kt):
                v_sl = v_buf[:, kt, h * (D + 1):(h + 1) * (D + 1)]
                for qc in range(2):
                    sl = slice(qc * 512, (qc + 1) * 512)
                    mm(av[qc][:D + 1, :], v_sl, pts_h[kt][:, sl],
                       start=(kt == 0), stop=(kt == NT - 1))

            def norm_chunk():
                dn = spool.tile([33, 512], F32, tag="dn", name=f"dn{h}")
                nc.vector.memset(dn[:], 1.0)
                nc.vector.tensor_copy(dn[0:1, :], av[0][D:D + 1, :])
                nc.vector.tensor_copy(dn[32:33, :], av[1][D:D + 1, :])
                rc = spool.tile([33, 512], F16, tag="rc", name=f"rc{h}")
                nc.vector.reciprocal(rc[:], dn[:])
                for qc in range(2):
                    sl = slice(qc * 512, (qc + 1) * 512)
                    r0 = 32 * qc
                    bc = ps_bg.tile([D, 512], F32, tag="bg",
                                    name=f"bc{h}_{qc}")
                    mm(bc[:], ones[r0:r0 + 1, :], rc[r0:r0 + 1, :],
                       start=True, stop=True)
                    attn_sl = attn_buf[p0:p0 + D, h // 2, sl]
                    nc.vector.tensor_copy(attn_sl, av[qc][:D, :])
                    nc.vector.tensor_tensor(attn_sl, attn_sl, bc[:], mult)

            backlog.extend(
                [lambda kt=kt: av_chunk(kt) for kt in range(NT)]
                + [norm_chunk])

        # V and the later head-pairs' QK run as backlog inside pair 0's loop
        backlog.extend([lambda nt=nt: v_chunk(nt) for nt in range(NT)])
        backlog.extend([lambda m=m, qc=qc: qk_chunk(m, qc)
                        for m in (1, 4) for qc in range(2)])

        # ---- attention, one head-pair at a time --------------------------
        for hp in range(HL // 2):
            mq, mk = hp, 3 + hp
            pts = [[None] * NT for _ in range(2)]
            for kt in range(NT):
                pss = []
                for par in range(2):
                    ps_st = ps_s.tile([P, NTOK], F32, tag="s",
                                      name=f"s{hp}_{par}_{kt}")
                    pss.append(ps_st)
                # interleave the two heads' matmuls -> concurrent row groups
                for qc in range(2):
                    sl = slice(qc * 512, (qc + 1) * 512)
                    for par in range(2):
                        p0 = D * par
                        k_sl = qk_buf[p0:p0 + D, mk, kt * P:(kt + 1) * P]
                        mm(pss[par][:, sl], k_sl, qk_buf[p0:p0 + D, mq, sl],
                           start=True, stop=True)
                for par in range(2):
                    pt = ptpool.tile([P, NTOK], F16, tag="ptk",
                                     name=f"pt{hp}_{par}_{kt}")
                    nc.scalar.activation(pt[:], pss[par][:], Exp, scale=SCALE)
                    pts[par][kt] = pt
                drain_backlog(3 if hp else 2)

            # queue this pair's AV/norm; drained during the next pair's loop
            av_head(2 * hp + 0, pts[0])
            av_head(2 * hp + 1, pts[1])
            if hp == 0:
                backlog.extend([lambda m=m, qc=qc: qk_chunk(m, qc)
                                for m in (2, 5) for qc in range(2)])

        while backlog:
            drain_backlog(len(backlog))

        # ---- output projection (row-parallel partial) --------------------
        for nt in range(NT):
            psy = ps_s.tile([P, NTOK], F32, tag="s", name=f"psy{nt}")
            for ks in range(3):
                for nsl in (slice(0, 512), slice(512, C)):
                    mm(psy[:, nsl], attn_buf[:, ks, nt * P:(nt + 1) * P],
                       wpj[:, ks, nsl], start=(ks == 0), stop=(ks == 2))
            ysb = ypool.tile([P, C], F32, tag="ysb", name=f"ysb{nt}")
            nc.vector.tensor_copy(ysb[:], psy[:, :C])
            nc.sync.dma_start(y_d[nt * P:(nt + 1) * P, :], ysb[:])


def _build():
    nc = bacc.Bacc("TRN2", target_bir_lowering=False, debug=False)
    xa_d = nc.dram_tensor("x_aug", [P, KS, NTOK], F16, kind="ExternalInput")
    wqk_d = nc.dram_tensor("w_qk", [P, KS, 2 * HL * D], F16, kind="ExternalInput")
    wv_d = nc.dram_tensor("w_v", [P, KS, HL * D], F16, kind="ExternalInput")
    wpj_d = nc.dram_tensor("w_pj", [P, 3, C], F16, kind="ExternalInput")
    y_d = nc.dram_tensor("y", [NTOK, C], F32, kind="ExternalOutput")
    with tile.TileContext(nc) as tc:
        _emit(nc, tc, xa_d.ap(), wqk_d.ap(), wv_d.ap(), wpj_d.ap(), y_d.ap())
    nc.compile()
    return nc


def get_nc():
    if "nc" not in _NC_CACHE:
        _NC_CACHE["nc"] = _build()
    return _NC_CACHE["nc"]


def make_core_inputs(x, w_qkv, b_qkv, w_proj, core):
    """Host-side shard prep for one core. core = 2*batch + head_group."""
    b, g = core // 2, core % 2
    hh0 = HL * g

    xa = np.ascontiguousarray(x[b].T).reshape(KS, P, NTOK).transpose(1, 0, 2)

    # column indices in w_qkv: col(h, d, s) = h*192 + d*3 + s
    hs = np.arange(hh0, hh0 + HL)
    ds_ = np.arange(D)
    qcols = (hs[:, None] * 192 + ds_[None, :] * 3 + 0).reshape(-1)
    kcols = qcols + 1
    vcols = qcols + 2
    assert np.all(np.asarray(b_qkv) == 0.0), \
        "kernel assumes zero qkv bias (guaranteed by the problem spec)"

    qk_cols = np.concatenate([qcols, kcols])           # 768 columns
    wqk = w_qkv[:, qk_cols].reshape(KS, P, -1).transpose(1, 0, 2)
    wv = w_qkv[:, vcols].reshape(KS, P, -1).transpose(1, 0, 2)
    wpj = w_proj[384 * g:384 * (g + 1)].reshape(3, P, C).transpose(1, 0, 2)

    return {"x_aug": np.ascontiguousarray(xa, dtype=np.float16),
            "w_qk": np.ascontiguousarray(wqk, dtype=np.float16),
            "w_v": np.ascontiguousarray(wv, dtype=np.float16),
            "w_pj": np.ascontiguousarray(wpj, dtype=np.float16)}


def run_sharded(x, w_qkv, b_qkv, w_proj, b_proj, trace=False, **kwargs):
    nc = get_nc()
    in_maps = [make_core_inputs(x, w_qkv, b_qkv, w_proj, c) for c in range(8)]
    res = run_bass_kernel_spmd(nc, in_maps, core_ids=list(range(8)),
                               trace=trace, **kwargs)
    B = x.shape[0]
    out = np.empty((B, NTOK, C), dtype=np.float32)
    for b in range(B):
        out[b] = res.results[2 * b]["y"] + res.results[2 * b + 1]["y"] \
            + b_proj[None, :]
    return out, res


def kernel(x, w_qkv, b_qkv, w_proj, b_proj, num_heads):
    x = np.asarray(x, dtype=np.float32)
    w_qkv = np.asarray(w_qkv, dtype=np.float32)
    b_qkv = np.asarray(b_qkv, dtype=np.float32)
    w_proj = np.asarray(w_proj, dtype=np.float32)
    b_proj = np.asarray(b_proj, dtype=np.float32)
    assert int(num_heads) == HFULL
    assert x.shape == (4, NTOK, C)
    out, _ = run_sharded(x, w_qkv, b_qkv, w_proj, b_proj)
    return out
